# revision 5
# baseline (speedup 1.0000x reference)
"""AttentionPool segment-softmax-pool kernel (v6) for 8 Trainium2 NeuronCores.

v5. Lessons from v3 (GpSimd ~1.5us/op: unusable) and v4 (ACT->DVE copy
chain serializes the block tail; DVE 2x perf modes never engage on this
toolchain, so routing h through ACT saves DVE nothing):

- z = e*h for ALL 4 heads in ONE DVE tensor_tensor per block, straight
  from PSUM (h slots contiguous in banks 4-7 -> [128, 8, 4, 64] AP).
  DVE does nothing else (~2.3us/block).
- one-hot DMA'd from DRAM as fp8 (host-built; fp8 lhsT x fp16 rhs
  scatter verified) - no iota/ycol, no is_equal.
- score bias c is PRELOADED into the score PSUM bank by a ones-matmul
  (lhsT=ones, rhs=c/128 replicated), the 8 score matmuls accumulate on
  top -> the DVE "+cvec" pass is gone.
- e = exp(leaky_relu(score)) entirely on ACT: Lrelu(alpha=0.2) then
  Exp, [128, 32] per block, writing z[..., 64] fp16.
- segment drains on ACT (Copy). scatter lag 14 as before.

Math identical to v2 (see kernel_v2.py docstring).
"""
import numpy as np

N_TOTAL = 500000
IN_CH = 128
OUT_CH = 64
NHEAD = 4
NUM_CLASSES = 1000
NEG_SLOPE = 0.2
NCORES = 8
ROWS_PER_CORE = N_TOTAL // NCORES          # 62500
NSEG = 8
TILES_PER_SEG = 64
SEG_ROWS = TILES_PER_SEG * 128             # 8192
TILES = NSEG * TILES_PER_SEG               # 512
ROWS = TILES * 128                         # 65536
TILES_PER_BLOCK = 8
NBLK = TILES // TILES_PER_BLOCK            # 64
LAG = 14

_prog_cache = {}


def _build():
    try:
        from concourse.compiler_utils import (get_compiler_flags,
                                              set_compiler_flags)
        set_compiler_flags([
            s.replace("--enable-ldw-opt=false", "--enable-ldw-opt=true")
            for s in get_compiler_flags()])
    except Exception:
        pass
    import concourse.bacc as bacc
    import concourse.mybir as mybir
    from concourse import tile

    f32 = mybir.dt.float32
    fp16 = mybir.dt.float16
    fp8 = mybir.dt.float8e4

    nc = bacc.Bacc(None, target_bir_lowering=False)

    xt_d = nc.dram_tensor("xt", [128, ROWS], fp16, kind="ExternalInput")
    oh_d = nc.dram_tensor("ohx", [128, ROWS], fp8, kind="ExternalInput")
    wvh_d = nc.dram_tensor("wvh", [128, 256], fp16, kind="ExternalInput")
    wvv_d = nc.dram_tensor("wvv", [128, 4], fp16, kind="ExternalInput")
    ones_d = nc.dram_tensor("ones", [128, 128], fp16, kind="ExternalInput")
    cv8_d = nc.dram_tensor("cv8", [128, 32], fp16, kind="ExternalInput")
    part_d = nc.dram_tensor("part", [1024, 260], f32, kind="ExternalOutput")

    ps = nc.alloc_psum_tensor("ps", [128, 4096], f32).ap()
    accum = [ps[:, 512 * b: 512 * b + 260] for b in range(3)]
    score_blk = ps[:, 512 * 3: 512 * 3 + 32]
    # 8 h slots, contiguous across banks 4-7: slot j = cols 2048+256j
    h_ps = [ps[:, 2048 + 256 * j: 2048 + 256 * (j + 1)] for j in range(8)]
    h_blk = ps[:, 2048: 4096].rearrange("p (j a b) -> p j a b", j=8, a=4)

    wvh_s = nc.alloc_sbuf_tensor("wvh_s", [128, 256], fp16).ap()
    wvv_s = nc.alloc_sbuf_tensor("wvv_s", [128, 4], fp16).ap()
    ones_s = nc.alloc_sbuf_tensor("ones_s", [128, 128], fp16).ap()
    cv8_s = nc.alloc_sbuf_tensor("cv8_s", [128, 32], fp16).ap()
    stage = nc.alloc_sbuf_tensor("stage", [128, NSEG, 260], f32).ap()

    mul = mybir.AluOpType.mult
    mx = mybir.AluOpType.max
    AF = mybir.ActivationFunctionType

    with tile.TileContext(nc) as tc:
        with (
            tc.tile_pool(name="io", bufs=3) as iop,
            tc.tile_pool(name="oh", bufs=3) as ohp,
            tc.tile_pool(name="zp", bufs=4) as zp,
            tc.tile_pool(name="sp", bufs=4) as sp,
        ):
            nc.sync.dma_start(wvh_s, wvh_d[:])
            nc.sync.dma_start(wvv_s, wvv_d[:])
            nc.sync.dma_start(ones_s, ones_d[:])
            nc.sync.dma_start(cv8_s, cv8_d[:])

            def scatter(t, ohb, z):
                s, j = t // TILES_PER_SEG, t % TILES_PER_BLOCK
                zi = z[:, j].rearrange("p a b -> p (a b)")
                nc.tensor.matmul(
                    accum[s % 3], ohb[:, j], zi,
                    start=(t % TILES_PER_SEG == 0),
                    stop=(t % TILES_PER_SEG == TILES_PER_SEG - 1),
                    skip_group_check=True)
                if t % TILES_PER_SEG == TILES_PER_SEG - 1:
                    nc.scalar.activation(stage[:, s], accum[s % 3], AF.Copy)

            def dma_xt(b):
                xt = iop.tile([128, 1024], fp16)
                nc.sync.dma_start(
                    xt[:], xt_d[:, b * 1024:(b + 1) * 1024])
                return xt

            def dma_oh(b):
                ohb = ohp.tile([128, 8, 128], fp8)
                nc.sync.dma_start(
                    ohb[:].rearrange("p j c -> p (j c)"),
                    oh_d[:, b * 1024:(b + 1) * 1024])
                return ohb

            def prep_preload():
                # c bias preload (poisons bank 3 only); per-tile score
                # matmuls accumulate on top with start=False.
                nc.tensor.matmul(
                    score_blk, ones_s, cv8_s,
                    start=True, stop=False, skip_group_check=True)

            def prep_score(xt, k):
                nc.tensor.matmul(
                    score_blk[:, 4 * k: 4 * k + 4],
                    xt[:, 128 * k: 128 * (k + 1)], wvv_s,
                    start=False, stop=(k == TILES_PER_BLOCK - 1),
                    skip_group_check=True)

            def prep_act():
                sc3 = sp.tile([128, 32], fp16)
                nc.scalar.activation(sc3[:], score_blk, AF.Prelu,
                                     alpha=NEG_SLOPE)
                z = zp.tile([128, TILES_PER_BLOCK, 4, 65], fp16)
                nc.scalar.activation(
                    z[:, :, :, 64],
                    sc3[:].rearrange("p (j a) -> p j a", j=8), AF.Exp)
                return z

            def prep_block(b, xt):
                prep_preload()
                for k in range(TILES_PER_BLOCK):
                    prep_score(xt, k)
                return prep_act()

            def z_quad(t, z):
                lo = (t % TILES_PER_BLOCK) - 3          # 0 or 4
                e = z[:, lo: lo + 4, :, 64]
                nc.vector.tensor_tensor(
                    z[:, lo: lo + 4, :, 0:64],
                    h_blk[:, lo: lo + 4],
                    e.broadcast_to([128, 4, 4, 64]),
                    mul)

            prev = []          # queue of (t, ohb, z) awaiting scatter
            oh_cur = None
            xts = {}
            z_cur = z_next = None
            for t in range(TILES):
                b, i = divmod(t, TILES_PER_BLOCK)
                if t == 0:
                    xts[0] = dma_xt(0)
                    oh_cur = dma_oh(0)
                    xts[1] = dma_xt(1)
                    xts[2] = dma_xt(2)
                    z_cur = prep_block(0, xts[0])
                elif i == 0:
                    z_cur = z_next
                    xts.pop(b - 1, None)
                    oh_cur = dma_oh(b)
                xt_cur = xts[b]
                nc.tensor.matmul(
                    h_ps[i], xt_cur[:, 128 * i: 128 * (i + 1)], wvh_s,
                    start=True, stop=True, skip_group_check=True)
                # spread next block's score matmuls one per iteration
                if b + 1 < NBLK:
                    if i == 0:
                        prep_preload()
                    prep_score(xts[b + 1], i)
                if i % 4 == 3:
                    z_quad(t, z_cur)
                    while len(prev) > LAG - 4:
                        scatter(*prev.pop(0))
                if i == 4 and b + 2 < NBLK:
                    xts[b + 2] = dma_xt(b + 2)
                if i == 7 and b + 1 < NBLK:
                    z_next = prep_act()
                prev.append((t, oh_cur, z_cur))
            for args in prev:
                scatter(*args)

            nc.sync.dma_start(
                part_d.rearrange("(j r) d -> r j d", r=128), stage)

    nc.compile()
    return nc


def _get_prog():
    if "p" not in _prog_cache:
        _prog_cache["p"] = _build()
    return _prog_cache["p"]


def _host_prep(x, y):
    """One core's shard -> device input map + per-segment class bases."""
    import ml_dtypes
    order = np.argsort(y, kind="stable")
    ys = y[order]
    counts = np.bincount(ys, minlength=NUM_CLASSES)
    class_starts = np.concatenate(([0], np.cumsum(counts)))
    seg_base = np.zeros(NSEG + 1, dtype=np.int64)
    c = 0
    for s in range(NSEG):
        seg_base[s] = c
        rows = 0
        ncls = 0
        while (c < NUM_CLASSES and ncls < 128
               and rows + counts[c] <= SEG_ROWS):
            rows += counts[c]
            c += 1
            ncls += 1
    assert c == NUM_CLASSES, "segment partition failed to cover classes"
    seg_base[NSEG] = NUM_CLASSES

    perm = np.full(ROWS, -1, dtype=np.int64)
    yrel = np.full(ROWS, -1, dtype=np.int32)
    for s in range(NSEG):
        lo_c, hi_c = seg_base[s], seg_base[s + 1]
        rlo, rhi = class_starts[lo_c], class_starts[hi_c]
        n = rhi - rlo
        perm[s * SEG_ROWS: s * SEG_ROWS + n] = order[rlo:rhi]
        yrel[s * SEG_ROWS: s * SEG_ROWS + n] = ys[rlo:rhi] - lo_c
    xt = np.zeros((128, ROWS), dtype=np.float16)
    valid = perm >= 0
    xt[:, valid] = np.ascontiguousarray(
        x[perm[valid]].T).astype(np.float16)
    # one-hot, fp8 0/1: ohx[p, t*128 + c] = (yrel[t*128+p] == c)
    yt = yrel.reshape(TILES, 128)                       # [t, p]
    oh = (yt[:, :, None] == np.arange(128)[None, None, :])   # [t, p, c]
    ohx = np.ascontiguousarray(
        oh.transpose(1, 0, 2).reshape(128, ROWS)).astype(
        ml_dtypes.float8_e4m3)
    return {"xt": xt, "ohx": ohx}, seg_base


def _host_weights(lin_w, lin_b, att_w, att_b):
    wvh = np.ascontiguousarray(lin_w.T).astype(np.float16)        # [128, 256]
    w3 = lin_w.reshape(NHEAD, OUT_CH, IN_CH).astype(np.float64)
    v = np.einsum("hjk,j->kh", w3, att_w[0].astype(np.float64))   # [128, 4]
    wvv = v.astype(np.float16)
    c = (lin_b.reshape(NHEAD, OUT_CH).astype(np.float64)
         @ att_w[0].astype(np.float64) + float(att_b[0]))          # [4]
    # c preload: ones[128,128] @ cv8[128,32] puts c[col] in every row
    cv8 = np.tile(np.tile((c / 128.0).astype(np.float16), 8), (128, 1))
    ones = np.ones((128, 128), dtype=np.float16)
    return {"wvh": wvh, "wvv": wvv, "ones": ones, "cv8": cv8}


def kernel(context_h_input, context_y, num_classes, lin_w, lin_b, att_w,
           att_b):
    from concourse.bass_utils import run_bass_kernel_spmd

    x = np.asarray(context_h_input, dtype=np.float32)
    y = np.asarray(context_y, dtype=np.int32)
    lin_w = np.asarray(lin_w, dtype=np.float32)
    lin_b = np.asarray(lin_b, dtype=np.float32)
    att_w = np.asarray(att_w, dtype=np.float32)
    att_b = np.asarray(att_b, dtype=np.float32)
    n = x.shape[0]
    assert int(num_classes) == NUM_CLASSES and n == N_TOTAL

    nc = _get_prog()
    wmap = _host_weights(lin_w, lin_b, att_w, att_b)
    in_maps = []
    bases = []
    for i in range(NCORES):
        lo, hi = i * ROWS_PER_CORE, (i + 1) * ROWS_PER_CORE
        m, seg_base = _host_prep(x[lo:hi], y[lo:hi])
        m.update(wmap)
        in_maps.append(m)
        bases.append(seg_base)

    res = run_bass_kernel_spmd(nc, in_maps, list(range(NCORES)))
    p = np.zeros((NUM_CLASSES, 260), dtype=np.float64)
    for seg_base, r in zip(bases, res.results):
        part = r["part"].astype(np.float64)
        for s in range(NSEG):
            lo_c, hi_c = seg_base[s], seg_base[s + 1]
            p[lo_c:hi_c] += part[128 * s: 128 * s + (hi_c - lo_c)]

    pc = p.reshape(NUM_CLASSES, NHEAD, 65)
    pooled = pc[:, :, 0:64]
    denom = pc[:, :, 64]
    out = pooled / denom[:, :, None] + lin_b.astype(np.float64).reshape(
        NHEAD, OUT_CH)[None]
    return out.reshape(NUM_CLASSES, NHEAD * OUT_CH).astype(np.float32)
